# revision 34
# baseline (speedup 1.0000x reference)
"""Trainium2 Bass kernel: MultiHeadAttention with QK-RMSNorm + partial rotary,
causal softmax. B=4, T=2048, D=1024, H=16, HD=64, fp32.

Sharding: 8 cores = 4 batches x 2 head-groups (8 heads each). Each core:
  - QKV projections for its batch, restricted to its 512 head-dims
  - causal attention for its 8 heads
  - partial output projection (its 512 contraction dims, all 1024 outputs)
Host sums the two head-group partials per batch (the all-reduce) and
transposes back.

v3 design: phase-interleaved, engine-specialized, single-exp scores.
  - Per 512-token block tt: QKV projections -> rotary+RMS-norm -> attention
    for i-block tt -> output projection of i-block tt. The ACT-heavy softmax
    and the DMA-heavy output store overlap the PE-heavy projections of the
    next block, keeping the PE dense so the HAM clock gate stays at K=8/8
    (fp32r at low clock is 4x slower).
  - K is normalized exactly like Q (bdiag matmul + sqrt + recip + mul), so
    exp needs no per-head scale and one [128,1024] exp op covers both heads
    of a pair (halves ACT instruction count - ACT is the attention pacer).
  - gpsimd runs ONLY partition_broadcast (softmax denominator): any second
    gpsimd ucode kernel would trigger ~6us IRAM reloads per switch.
  - ACT tables: {Square,Sqrt,Copy} + {Exp,Copy} - two loads per block max.
  - V, softmax weights, attention output and wo in bf16 (same PE rate as
    fp32r, halves SBUF/DVE cost, dodges fp32r small-N penalty on diagonal).
  - causal diagonal blocks compute/exp only the valid column strip; mask is
    an additive -300 on scores pre-exp.
"""

import numpy as np
from contextlib import ExitStack

import concourse.bass as bass
import concourse.tile as tile
import concourse.mybir as mybir
from concourse import bacc

F32 = mybir.dt.float32
BF16 = mybir.dt.bfloat16
MM_DT = mybir.dt.float32r
AF = mybir.ActivationFunctionType

D = 1024   # model dim
DH = 512   # head-group width per core (8 heads x 64)
NH = 8     # heads per core
HD = 64    # head dim
NKC = D // 128   # k-chunks over model dim
EPS = 1e-6


def _r(ap):
    return ap.bitcast(MM_DT)


def build_kernel(nc: bass.Bass, T: int = 2048):
    NTT = T // 512     # 512-wide t/i blocks
    NTS = T // 128     # 128-wide t/j chunks

    xt = nc.dram_tensor("xt", [D, T], F32, kind="ExternalInput").ap()
    wqt = nc.dram_tensor("wqt", [D, DH], F32, kind="ExternalInput").ap()
    wkt = nc.dram_tensor("wkt", [D, DH], F32, kind="ExternalInput").ap()
    wvt = nc.dram_tensor("wvt", [D, DH], F32, kind="ExternalInput").ap()
    wot = nc.dram_tensor("wot", [DH, D], BF16, kind="ExternalInput").ap()
    c2d = nc.dram_tensor("c2", [128, T], F32, kind="ExternalInput").ap()
    s2d = nc.dram_tensor("s2", [128, T], F32, kind="ExternalInput").ap()
    pswapd = nc.dram_tensor("pswap", [128, 128], F32, kind="ExternalInput").ap()
    bdiagd = nc.dram_tensor("bdiag", [128, 128], F32, kind="ExternalInput").ap()
    trid = nc.dram_tensor("trimask", [128, 256], F32, kind="ExternalInput").ap()
    yt = nc.dram_tensor("yt", [D, T], BF16, kind="ExternalOutput").ap()

    with tile.TileContext(nc) as tc, ExitStack() as ctx:
        # ---- persistent SBUF pools --------------------------------------
        qk_pool = ctx.enter_context(tc.tile_pool(name="qk", bufs=1))
        v_pool = ctx.enter_context(tc.tile_pool(name="v", bufs=1))
        ot_pool = ctx.enter_context(tc.tile_pool(name="otf", bufs=1))
        const_pool = ctx.enter_context(tc.tile_pool(name="const", bufs=1))
        w_pool = ctx.enter_context(tc.tile_pool(name="wqkv", bufs=1))
        wo_pool = ctx.enter_context(tc.tile_pool(name="wo", bufs=1))
        x_pool = ctx.enter_context(tc.tile_pool(name="xs", bufs=8))
        cs_pool = ctx.enter_context(tc.tile_pool(name="cs", bufs=1))
        t_pool = ctx.enter_context(tc.tile_pool(name="rott", bufs=2))
        p_pool = ctx.enter_context(tc.tile_pool(name="pexp", bufs=4))
        e_pool = ctx.enter_context(tc.tile_pool(name="epi", bufs=1))
        st_pool = ctx.enter_context(tc.tile_pool(name="stg3", bufs=2))
        # single PSUM pool; tags share the 8 banks across phases:
        #   A: proj accum [128,512] + scores pair [128,1024]   (2x2 banks)
        #   B: rotary pswap out + attn AV accum                (2 banks)
        #   C: rotary bdiag out + out-proj accum               (2 banks)
        ps = ctx.enter_context(tc.tile_pool(name="ps", bufs=1, space="PSUM"))

        qt_s = [qk_pool.tile([128, T], F32, name=f"qt{j}") for j in range(4)]
        kt_s = [qk_pool.tile([128, T], F32, name=f"kt{j}") for j in range(4)]
        v_s = [v_pool.tile([128, NH * 65], BF16, name=f"vt{j}")
               for j in range(NTS)]
        otf = [ot_pool.tile([128, T], BF16, name=f"otf{j}") for j in range(4)]

        # weights first: they gate the first projection matmuls.
        wq_s = [w_pool.tile([128, DH], F32, name=f"wq{k}") for k in range(NKC)]
        wk_s = [w_pool.tile([128, DH], F32, name=f"wk{k}") for k in range(NKC)]
        wv_s = [w_pool.tile([128, DH], F32, name=f"wv{k}") for k in range(NKC)]
        for k in range(NKC):
            ksl = slice(k * 128, (k + 1) * 128)
            eng = nc.sync if k % 2 == 0 else nc.scalar
            eng.dma_start(_r(wq_s[k][:]), _r(wqt[ksl, :]))
        pswap = const_pool.tile([128, 128], F32, name="pswap_s")
        bdiag = const_pool.tile([128, 128], F32, name="bdiag_s")
        trif = const_pool.tile([128, 256], F32, name="trif_s")
        nc.scalar.dma_start(_r(pswap[:]), _r(pswapd[:]))
        nc.scalar.dma_start(_r(bdiag[:]), _r(bdiagd[:]))
        for k in range(NKC):
            ksl = slice(k * 128, (k + 1) * 128)
            nc.scalar.dma_start(_r(wk_s[k][:]), _r(wkt[ksl, :]))
            nc.scalar.dma_start(_r(wv_s[k][:]), _r(wvt[ksl, :]))
        nc.scalar.dma_start(trif[:], trid[:])
        wob = [wo_pool.tile([128, D], BF16, name=f"wob{k}") for k in range(4)]
        for k4 in range(4):
            nc.scalar.dma_start(wob[k4][:], wot[k4 * 128:(k4 + 1) * 128, :])
        epsb = const_pool.tile([128, 1], F32, name="epsb")
        nc.vector.memset(epsb[:], 8.0 * EPS)
        onescb = const_pool.tile([128, NH], BF16, name="onescb")
        nc.vector.memset(onescb[:], 1.0)

        # ---------------- emission helpers ------------------------------
        def emit_x_dma(tt):
            tsl = slice(tt * 512, (tt + 1) * 512)
            xts = []
            for k in range(NKC):
                xc = x_pool.tile([128, 512], F32, name="xc", tag="xc")
                nc.gpsimd.dma_start(_r(xc[:]),
                                    _r(xt[k * 128:(k + 1) * 128, tsl]))
                xts.append(xc)
            return xts

        def py_chain(pib, dt_):
            """Out-projection of one dout chunk of i-block pib (PE filler)."""
            dsl = slice(dt_ * 128, (dt_ + 1) * 128)
            psl = slice(pib * 512, (pib + 1) * 512)
            py = ps.tile([128, 512], F32, name="py", tag="C", bufs=2)
            for k4 in range(4):
                nc.tensor.matmul(py[:], wob[k4][:, dsl], otf[k4][:, psl],
                                 start=(k4 == 0), stop=(k4 == 3))
            st = st_pool.tile([128, 512], BF16, name="st", tag="st", bufs=3)
            nc.vector.tensor_copy(st[:], py[:])
            nc.sync.dma_start(yt[dsl, psl], st[:])

        def proj_closures(tt, xts, act_copies):
            """12 PE projection k-groups for block tt (q/k/v); copies
            trail on ACT early (slack) or DVE late (exp latency)."""
            tsl = slice(tt * 512, (tt + 1) * 512)
            out = []
            for (wsrc, dsts) in ((wq_s, qt_s), (wk_s, kt_s)):
                for hp in range(4):
                    def g(wsrc=wsrc, dsts=dsts, hp=hp):
                        jsl = slice(hp * 128, (hp + 1) * 128)
                        pp = ps.tile([128, 1024], F32, name="pp", tag="A",
                                     bufs=2)
                        for k in range(NKC):
                            nc.tensor.matmul(
                                pp[:, 0:512], _r(wsrc[k][:, jsl]),
                                _r(xts[k][:]),
                                start=(k == 0), stop=(k == NKC - 1))
                        if act_copies:
                            nc.scalar.copy(_r(dsts[hp][:, tsl]), pp[:, 0:512])
                        else:
                            nc.vector.tensor_copy(_r(dsts[hp][:, tsl]),
                                                  pp[:, 0:512])
                    out.append(g)
            for ts_ in range(4):
                def g(ts_=ts_):
                    ci = tt * 4 + ts_
                    pv = ps.tile([128, 1024], F32, name="pv", tag="A", bufs=2)
                    for k in range(NKC):
                        nc.tensor.matmul(
                            pv[:, 0:512],
                            _r(xts[k][:, ts_ * 128:(ts_ + 1) * 128]),
                            _r(wv_s[k][:]),
                            start=(k == 0), stop=(k == NKC - 1))
                    v3 = v_s[ci].rearrange("p (h e) -> p h e", h=NH)
                    if act_copies:
                        nc.scalar.copy(
                            v3[:, :, 0:64],
                            pv[:, 0:512].rearrange("p (h e) -> p h e", h=NH))
                    else:
                        nc.vector.tensor_copy(
                            v3[:, :, 0:64],
                            pv[:, 0:512].rearrange("p (h e) -> p h e", h=NH))
                    nc.vector.tensor_copy(v3[:, :, 64:65],
                                          onescb[:].unsqueeze(-1))
                out.append(g)
            return out

        def rot_closures(tt):
            """Rotary + RMS-norm for q and k of block tt. Squares on DVE,
            sqrt on ACT (Square/Sqrt/Copy + Exp/Copy = 2 tables; squares
            and sqrts batch per block so table swaps stay ~2/block)."""
            tsl = slice(tt * 512, (tt + 1) * 512)
            c2t = cs_pool.tile([128, 512], F32, name="c2t", tag="c2t")
            s2t = cs_pool.tile([128, 512], F32, name="s2t", tag="s2t")
            nc.sync.dma_start(c2t[:], c2d[:, tsl])
            nc.sync.dma_start(s2t[:], s2d[:, tsl])
            out = []
            for hp in range(4):
                for nm in ("q", "k"):
                    def g(hp=hp, nm=nm):
                        dst = (qt_s if nm == "q" else kt_s)[hp]
                        sq = t_pool.tile([128, 512], F32, name="sq",
                                         tag=f"sq{nm}")
                        nc.vector.scalar_tensor_tensor(
                            _r(sq[:]), dst[:, tsl], 1.0, dst[:, tsl],
                            mybir.AluOpType.mult, mybir.AluOpType.mult)
                        xs_ = ps.tile([128, 512], F32, name="xs", tag="B",
                                      bufs=2)
                        nc.tensor.matmul(xs_[:], _r(pswap[:]),
                                         _r(dst[:, tsl]),
                                         start=True, stop=True)
                        ms = ps.tile([128, 512], F32, name="ms", tag="C",
                                     bufs=2)
                        nc.tensor.matmul(ms[:], _r(bdiag[:]), _r(sq[:]),
                                         start=True, stop=True)
                        s1 = t_pool.tile([128, 512], F32, name="s1",
                                         tag=f"s1{nm}")
                        nc.scalar.activation(s1[:], ms[:], AF.Sqrt,
                                             scale=0.125, bias=epsb[:])
                        nc.vector.reciprocal_approx_fast(out=s1[:], in_=s1[:])
                        nc.vector.tensor_mul(_r(dst[:, tsl]), dst[:, tsl],
                                             c2t[:])
                        nc.vector.tensor_mul(xs_[:], xs_[:], s2t[:])
                        nc.vector.tensor_add(_r(dst[:, tsl]), dst[:, tsl],
                                             xs_[:])
                        nc.vector.tensor_mul(_r(dst[:, tsl]), dst[:, tsl],
                                             s1[:])
                    out.append(g)
            return out

        def attn_closures(tt):
            """Software-pipelined attention for i-block tt: scores(jt+1) is
            emitted before AV(jt) so interleaved PE work covers exp."""
            ib = tt
            isl = slice(tt * 512, (tt + 1) * 512)
            nj = 4 * ib + 4
            late = tt >= 2

            def emit_sc(hp, jt):
                jsl = slice(jt * 128, (jt + 1) * 128)
                c_ = jt - 4 * ib          # >=0 on diagonal chunks
                off = 128 * c_ if c_ >= 0 else 0
                osc = off if off <= 256 else 256   # keep f32r N>=256
                sc = ps.tile([128, 1024], F32, name="sc", tag="A", bufs=2)
                for h2 in range(2):
                    ho = h2 * 64
                    nc.tensor.matmul(
                        sc[:, 512 * h2 + osc:512 * h2 + 512],
                        _r(kt_s[hp][ho:ho + 64, jsl]),
                        _r(qt_s[hp][ho:ho + 64,
                                    ib * 512 + osc:ib * 512 + 512]),
                        start=True, stop=True)
                return sc, off, c_

            def warm_mm(box, n):
                # keep-warm padding: bf16 matmuls into unused psum rows
                # 96-127 of the AV accumulator; ~213ns each, they hold the
                # HAM clock gate at K=8/8 while ACT paces the softmax.
                for _ in range(n):
                    # start/stop False: ride the open AV accumulation group
                    nc.tensor.matmul(box["ot"][0][96:128, 0:512],
                                     v_s[0][:, 0:32], v_s[0][:, 0:512],
                                     start=False, stop=False,
                                     tile_position=(0, 96))

            def emit_av(hp, box, jt, p, off):
                for h2 in range(2):
                    h = 2 * hp + h2
                    nc.tensor.matmul(
                        box["ot"][h2][0:65, off:512],
                        v_s[jt][:, 65 * h:65 * h + 65],
                        p[:, 512 * h2 + off:512 * h2 + 512],
                        start=(jt == 0), stop=(jt == nj - 1))

            out = []
            for hp in range(4):
                box = {}

                def c_start(hp=hp, box=box):
                    box["ot"] = [ps.tile([128, 512], F32, name="otp",
                                         tag="B", bufs=2) for _ in range(2)]
                    box["nxt"] = emit_sc(hp, 0)
                    box["avq"] = []
                out.append(c_start)

                for jt in range(nj):
                    def c_item(hp=hp, jt=jt, box=box):
                        sc, off, c_ = box["nxt"]
                        sc3 = sc.rearrange("p (h e) -> p h e", h=2)
                        if c_ >= 0:
                            # additive causal mask (-300 below diag) pre-exp
                            nc.vector.tensor_add(
                                sc3[:, :, off:off + 128],
                                sc3[:, :, off:off + 128],
                                trif[:].rearrange("p (h e) -> p h e", h=2))
                        p = p_pool.tile([128, 1024], BF16, name="p", tag="p")
                        p3 = p.rearrange("p (h e) -> p h e", h=2)
                        nc.scalar.activation(p3[:, :, off:512],
                                             sc3[:, :, off:512], AF.Exp)
                        if jt + 1 < nj:
                            box["nxt"] = emit_sc(hp, jt + 1)
                        # AV runs one pipeline step behind its exp so the
                        # PE never waits on the ACT queue.
                        box["avq"].append((jt, p, off))
                        if len(box["avq"]) > 1:
                            emit_av(hp, box, *box["avq"].pop(0))
                            warm_mm(box, 2 if tt == NTT - 1 else 1)
                    out.append(c_item)

                def c_epi(hp=hp, box=box):
                    while box["avq"]:
                        emit_av(hp, box, *box["avq"].pop(0))
                    # epilogue: divide by the denominator row (psum row 64).
                    # partition_broadcast is the ONLY gpsimd ucode kernel in
                    # the program, so its IRAM load is paid once.
                    for h2 in range(2):
                        op = box["ot"][h2]
                        den = e_pool.tile([1, 512], F32, name="den",
                                          tag="den")
                        nc.vector.tensor_copy(den[:], op[64:65, :])
                        rden = e_pool.tile([1, 512], F32, name="rden",
                                           tag="rden")
                        nc.vector.reciprocal_approx_fast(out=rden[:],
                                                         in_=den[:])
                        rb = e_pool.tile([64, 512], F32, name="rb", tag="rb")
                        nc.gpsimd.partition_broadcast(rb[:], rden[:],
                                                      channels=64)
                        ho = h2 * 64
                        nc.vector.tensor_mul(otf[hp][ho:ho + 64, isl],
                                             op[0:64, :], rb[:])
                out.append(c_epi)
            return out

        def interleave(a, b):
            ia = ib_ = 0
            while ia < len(a) or ib_ < len(b):
                if ib_ >= len(b) or (ia < len(a)
                                     and ia * len(b) <= ib_ * len(a)):
                    a[ia]()
                    ia += 1
                else:
                    b[ib_]()
                    ib_ += 1

        # ---------------- schedule --------------------------------------
        xts = emit_x_dma(0)
        for f in proj_closures(0, xts, True):
            f()
        pending_py = []
        for tt in range(NTT):
            a_items = rot_closures(tt) + attn_closures(tt)
            b_items = []
            if tt + 1 < NTT:
                xts = emit_x_dma(tt + 1)
                b_items += proj_closures(tt + 1, xts, tt + 1 < NTT - 1)
            if tt == NTT - 1:
                take = [p_ for p_ in pending_py if p_[0] <= tt - 1]
            else:
                take = [p_ for p_ in pending_py if p_[0] == tt - 2]
            for p_ in take:
                pending_py.remove(p_)
                b_items.append(lambda p_=p_: py_chain(*p_))
            interleave(a_items, b_items)
            pending_py += [(tt, d) for d in range(8)]
        for pib, d in pending_py:
            py_chain(pib, d)
    return nc


# ---------------- host-side tables & shard prep -------------------------

def host_tables(T: int = 2048):
    n = HD // 4
    af = (1.0 / 1024) ** np.linspace(0, 1, n, dtype=np.float32)
    af = np.concatenate([af, np.zeros(n, np.float32)])  # [32]
    theta = np.outer(np.arange(T, dtype=np.float32), af)  # [T, 32]
    cosT = np.cos(theta).T.astype(np.float32)  # [32, T]
    sinT = np.sin(theta).T.astype(np.float32)
    c2 = np.tile(cosT, (4, 1))                             # [128, T]
    s2 = np.tile(np.concatenate([sinT, -sinT], 0), (2, 1))  # [128, T]
    km = np.arange(128)
    pswap = (km[:, None] == (km[None, :] ^ 32)).astype(np.float32)
    bdiag = ((km[:, None] // 64) == (km[None, :] // 64)).astype(np.float32)
    r_ = np.arange(128)[:, None]
    c_ = np.arange(128)[None, :]
    tri1 = np.where(c_ >= r_, 0.0, -300.0).astype(np.float32)
    tri = np.concatenate([tri1, tri1], axis=1)             # [128, 256]
    return {"c2": np.ascontiguousarray(c2), "s2": np.ascontiguousarray(s2),
            "pswap": pswap, "bdiag": bdiag,
            "trimask": np.ascontiguousarray(tri)}


def _bf16():
    import ml_dtypes
    return ml_dtypes.bfloat16


def core_inputs(x, wq, wk, wv, wo, core: int, T: int = 2048):
    b, g = core % 4, core // 4
    sl = slice(g * DH, (g + 1) * DH)
    m = {
        "xt": np.ascontiguousarray(np.asarray(x[b]).T.astype(np.float32)),
        "wqt": np.ascontiguousarray(np.asarray(wq)[sl, :].T.astype(np.float32)),
        "wkt": np.ascontiguousarray(np.asarray(wk)[sl, :].T.astype(np.float32)),
        "wvt": np.ascontiguousarray(np.asarray(wv)[sl, :].T.astype(np.float32)),
        "wot": np.ascontiguousarray(
            np.asarray(wo)[:, sl].T.astype(_bf16())),
    }
    m.update(host_tables(T))
    return m


_CACHE = {}


def _get_nc(T: int = 2048):
    key = ("nc", T)
    if key not in _CACHE:
        nc = bacc.Bacc("TRN2", target_bir_lowering=False, debug=False)
        build_kernel(nc, T)
        nc.compile()
        _CACHE[key] = nc
    return _CACHE[key]


def kernel(x, wq, wk, wv, wo, mask=None):
    from concourse import bass_utils
    nc = _get_nc(2048)
    in_maps = [core_inputs(x, wq, wk, wv, wo, c) for c in range(8)]
    res = bass_utils.run_bass_kernel_spmd(nc, in_maps, list(range(8)))
    outs = [np.asarray(res.results[c]["yt"]).astype(np.float32)
            for c in range(8)]
    out = np.empty((4, 2048, 1024), np.float32)
    for b in range(4):
        out[b] = (outs[b] + outs[b + 4]).T
    return out


# revision 37
# speedup vs baseline: 1.0056x; 1.0056x over previous
"""Trainium2 Bass kernel: MultiHeadAttention with QK-RMSNorm + partial rotary,
causal softmax. B=4, T=2048, D=1024, H=16, HD=64, fp32.

Sharding: 8 cores = 4 batches x 2 head-groups (8 heads each). Each core:
  - QKV projections for its batch, restricted to its 512 head-dims
  - causal attention for its 8 heads
  - partial output projection (its 512 contraction dims, all 1024 outputs)
Host sums the two head-group partials per batch (the all-reduce) and
transposes back.

v3 design: phase-interleaved, engine-specialized, single-exp scores.
  - Per 512-token block tt: QKV projections -> rotary+RMS-norm -> attention
    for i-block tt -> output projection of i-block tt. The ACT-heavy softmax
    and the DMA-heavy output store overlap the PE-heavy projections of the
    next block, keeping the PE dense so the HAM clock gate stays at K=8/8
    (fp32r at low clock is 4x slower).
  - K is normalized exactly like Q (bdiag matmul + sqrt + recip + mul), so
    exp needs no per-head scale and one [128,1024] exp op covers both heads
    of a pair (halves ACT instruction count - ACT is the attention pacer).
  - gpsimd runs ONLY partition_broadcast (softmax denominator): any second
    gpsimd ucode kernel would trigger ~6us IRAM reloads per switch.
  - ACT tables: {Square,Sqrt,Copy} + {Exp,Copy} - two loads per block max.
  - V, softmax weights, attention output and wo in bf16 (same PE rate as
    fp32r, halves SBUF/DVE cost, dodges fp32r small-N penalty on diagonal).
  - causal diagonal blocks compute/exp only the valid column strip; mask is
    an additive -300 on scores pre-exp.
"""

import numpy as np
from contextlib import ExitStack

import concourse.bass as bass
import concourse.tile as tile
import concourse.mybir as mybir
from concourse import bacc

F32 = mybir.dt.float32
BF16 = mybir.dt.bfloat16
MM_DT = mybir.dt.float32r
AF = mybir.ActivationFunctionType

D = 1024   # model dim
DH = 512   # head-group width per core (8 heads x 64)
NH = 8     # heads per core
HD = 64    # head dim
NKC = D // 128   # k-chunks over model dim
EPS = 1e-6


def _r(ap):
    return ap.bitcast(MM_DT)


def build_kernel(nc: bass.Bass, T: int = 2048):
    NTT = T // 512     # 512-wide t/i blocks
    NTS = T // 128     # 128-wide t/j chunks

    xt = nc.dram_tensor("xt", [D, T], F32, kind="ExternalInput").ap()
    wqt = nc.dram_tensor("wqt", [D, DH], F32, kind="ExternalInput").ap()
    wkt = nc.dram_tensor("wkt", [D, DH], F32, kind="ExternalInput").ap()
    wvt = nc.dram_tensor("wvt", [D, DH], F32, kind="ExternalInput").ap()
    wot = nc.dram_tensor("wot", [DH, D], BF16, kind="ExternalInput").ap()
    c2d = nc.dram_tensor("c2", [128, T], F32, kind="ExternalInput").ap()
    s2d = nc.dram_tensor("s2", [128, T], F32, kind="ExternalInput").ap()
    pswapd = nc.dram_tensor("pswap", [128, 128], F32, kind="ExternalInput").ap()
    bdiagd = nc.dram_tensor("bdiag", [128, 128], F32, kind="ExternalInput").ap()
    trid = nc.dram_tensor("trimask", [128, 256], F32, kind="ExternalInput").ap()
    yt = nc.dram_tensor("yt", [D, T], BF16, kind="ExternalOutput").ap()

    with tile.TileContext(nc) as tc, ExitStack() as ctx:
        # ---- persistent SBUF pools --------------------------------------
        qk_pool = ctx.enter_context(tc.tile_pool(name="qk", bufs=1))
        v_pool = ctx.enter_context(tc.tile_pool(name="v", bufs=1))
        ot_pool = ctx.enter_context(tc.tile_pool(name="otf", bufs=1))
        const_pool = ctx.enter_context(tc.tile_pool(name="const", bufs=1))
        w_pool = ctx.enter_context(tc.tile_pool(name="wqkv", bufs=1))
        wo_pool = ctx.enter_context(tc.tile_pool(name="wo", bufs=1))
        x_pool = ctx.enter_context(tc.tile_pool(name="xs", bufs=8))
        cs_pool = ctx.enter_context(tc.tile_pool(name="cs", bufs=1))
        t_pool = ctx.enter_context(tc.tile_pool(name="rott", bufs=2))
        p_pool = ctx.enter_context(tc.tile_pool(name="pexp", bufs=4))
        e_pool = ctx.enter_context(tc.tile_pool(name="epi", bufs=1))
        st_pool = ctx.enter_context(tc.tile_pool(name="stg3", bufs=2))
        # single PSUM pool; tags share the 8 banks across phases:
        #   A: proj accum [128,512] + scores pair [128,1024]   (2x2 banks)
        #   B: rotary pswap out + attn AV accum                (2 banks)
        #   C: rotary bdiag out + out-proj accum               (2 banks)
        ps = ctx.enter_context(tc.tile_pool(name="ps", bufs=1, space="PSUM"))

        qt_s = [qk_pool.tile([128, T], F32, name=f"qt{j}") for j in range(4)]
        kt_s = [qk_pool.tile([128, T], F32, name=f"kt{j}") for j in range(4)]
        v_s = [v_pool.tile([128, NH * 65], BF16, name=f"vt{j}")
               for j in range(NTS)]
        otf = [ot_pool.tile([128, T], BF16, name=f"otf{j}") for j in range(4)]

        # weights first: they gate the first projection matmuls.
        wq_s = [w_pool.tile([128, DH], F32, name=f"wq{k}") for k in range(NKC)]
        wk_s = [w_pool.tile([128, DH], F32, name=f"wk{k}") for k in range(NKC)]
        wv_s = [w_pool.tile([128, DH], F32, name=f"wv{k}") for k in range(NKC)]
        for k in range(NKC):
            ksl = slice(k * 128, (k + 1) * 128)
            eng = nc.sync if k % 2 == 0 else nc.scalar
            eng.dma_start(_r(wq_s[k][:]), _r(wqt[ksl, :]))
        pswap = const_pool.tile([128, 128], F32, name="pswap_s")
        bdiag = const_pool.tile([128, 128], F32, name="bdiag_s")
        trif = const_pool.tile([128, 256], F32, name="trif_s")
        nc.scalar.dma_start(_r(pswap[:]), _r(pswapd[:]))
        nc.scalar.dma_start(_r(bdiag[:]), _r(bdiagd[:]))
        for k in range(NKC):
            ksl = slice(k * 128, (k + 1) * 128)
            nc.scalar.dma_start(_r(wk_s[k][:]), _r(wkt[ksl, :]))
            nc.scalar.dma_start(_r(wv_s[k][:]), _r(wvt[ksl, :]))
        nc.scalar.dma_start(trif[:], trid[:])
        wob = [wo_pool.tile([128, D], BF16, name=f"wob{k}") for k in range(4)]
        for k4 in range(4):
            nc.scalar.dma_start(wob[k4][:], wot[k4 * 128:(k4 + 1) * 128, :])
        epsb = const_pool.tile([128, 1], F32, name="epsb")
        nc.vector.memset(epsb[:], 8.0 * EPS)
        onescb = const_pool.tile([128, NH], BF16, name="onescb")
        nc.vector.memset(onescb[:], 1.0)

        # ---------------- emission helpers ------------------------------
        def emit_x_dma(tt):
            tsl = slice(tt * 512, (tt + 1) * 512)
            xts = []
            for k in range(NKC):
                xc = x_pool.tile([128, 512], F32, name="xc", tag="xc")
                nc.gpsimd.dma_start(_r(xc[:]),
                                    _r(xt[k * 128:(k + 1) * 128, tsl]))
                xts.append(xc)
            return xts

        def py_chain(pib, dt_):
            """Out-projection of one dout chunk of i-block pib (PE filler)."""
            dsl = slice(dt_ * 128, (dt_ + 1) * 128)
            psl = slice(pib * 512, (pib + 1) * 512)
            py = ps.tile([128, 512], F32, name="py", tag="C", bufs=2)
            for k4 in range(4):
                nc.tensor.matmul(py[:], wob[k4][:, dsl], otf[k4][:, psl],
                                 start=(k4 == 0), stop=(k4 == 3))
            st = st_pool.tile([128, 512], BF16, name="st", tag="st", bufs=3)
            nc.vector.tensor_copy(st[:], py[:])
            nc.sync.dma_start(yt[dsl, psl], st[:])

        def proj_closures(tt, xts, act_copies):
            """12 PE projection k-groups for block tt (q/k/v); copies
            trail on ACT early (slack) or DVE late (exp latency)."""
            tsl = slice(tt * 512, (tt + 1) * 512)
            out = []
            for (wsrc, dsts) in ((wq_s, qt_s), (wk_s, kt_s)):
                for hp in range(4):
                    def g(wsrc=wsrc, dsts=dsts, hp=hp):
                        jsl = slice(hp * 128, (hp + 1) * 128)
                        pp = ps.tile([128, 1024], F32, name="pp", tag="A",
                                     bufs=2)
                        for k in range(NKC):
                            nc.tensor.matmul(
                                pp[:, 0:512], _r(wsrc[k][:, jsl]),
                                _r(xts[k][:]),
                                start=(k == 0), stop=(k == NKC - 1))
                        if act_copies:
                            nc.scalar.copy(_r(dsts[hp][:, tsl]), pp[:, 0:512])
                        else:
                            nc.vector.tensor_copy(_r(dsts[hp][:, tsl]),
                                                  pp[:, 0:512])
                    out.append(g)
            for ts_ in range(4):
                def g(ts_=ts_):
                    ci = tt * 4 + ts_
                    pv = ps.tile([128, 1024], F32, name="pv", tag="A", bufs=2)
                    for k in range(NKC):
                        nc.tensor.matmul(
                            pv[:, 0:512],
                            _r(xts[k][:, ts_ * 128:(ts_ + 1) * 128]),
                            _r(wv_s[k][:]),
                            start=(k == 0), stop=(k == NKC - 1))
                    v3 = v_s[ci].rearrange("p (h e) -> p h e", h=NH)
                    if act_copies:
                        nc.scalar.copy(
                            v3[:, :, 0:64],
                            pv[:, 0:512].rearrange("p (h e) -> p h e", h=NH))
                    else:
                        nc.vector.tensor_copy(
                            v3[:, :, 0:64],
                            pv[:, 0:512].rearrange("p (h e) -> p h e", h=NH))
                    nc.vector.tensor_copy(v3[:, :, 64:65],
                                          onescb[:].unsqueeze(-1))
                out.append(g)
            return out

        def rot_closures(tt):
            """Rotary + RMS-norm for q and k of block tt. Squares on DVE,
            sqrt on ACT (Square/Sqrt/Copy + Exp/Copy = 2 tables; squares
            and sqrts batch per block so table swaps stay ~2/block)."""
            tsl = slice(tt * 512, (tt + 1) * 512)
            c2t = cs_pool.tile([128, 512], F32, name="c2t", tag="c2t")
            s2t = cs_pool.tile([128, 512], F32, name="s2t", tag="s2t")
            nc.sync.dma_start(c2t[:], c2d[:, tsl])
            nc.sync.dma_start(s2t[:], s2d[:, tsl])
            out = []
            for hp in range(4):
                for nm in ("q", "k"):
                    def g(hp=hp, nm=nm):
                        dst = (qt_s if nm == "q" else kt_s)[hp]
                        sq = t_pool.tile([128, 512], F32, name="sq",
                                         tag=f"sq{nm}")
                        nc.vector.scalar_tensor_tensor(
                            _r(sq[:]), dst[:, tsl], 1.0, dst[:, tsl],
                            mybir.AluOpType.mult, mybir.AluOpType.mult)
                        xs_ = ps.tile([128, 512], F32, name="xs", tag="B",
                                      bufs=2)
                        nc.tensor.matmul(xs_[:], _r(pswap[:]),
                                         _r(dst[:, tsl]),
                                         start=True, stop=True)
                        ms = ps.tile([128, 512], F32, name="ms", tag="C",
                                     bufs=2)
                        nc.tensor.matmul(ms[:], _r(bdiag[:]), _r(sq[:]),
                                         start=True, stop=True)
                        s1 = t_pool.tile([128, 512], F32, name="s1",
                                         tag=f"s1{nm}")
                        nc.scalar.activation(s1[:], ms[:], AF.Sqrt,
                                             scale=0.125, bias=epsb[:])
                        nc.vector.reciprocal_approx_fast(out=s1[:], in_=s1[:])
                        nc.vector.tensor_mul(_r(dst[:, tsl]), dst[:, tsl],
                                             c2t[:])
                        nc.vector.tensor_mul(xs_[:], xs_[:], s2t[:])
                        nc.vector.tensor_add(_r(dst[:, tsl]), dst[:, tsl],
                                             xs_[:])
                        nc.vector.tensor_mul(_r(dst[:, tsl]), dst[:, tsl],
                                             s1[:])
                    out.append(g)
            return out

        def attn_closures(tt):
            """Software-pipelined attention for i-block tt: scores(jt+1) is
            emitted before AV(jt) so interleaved PE work covers exp."""
            ib = tt
            isl = slice(tt * 512, (tt + 1) * 512)
            nj = 4 * ib + 4
            late = tt >= 2

            def emit_sc(hp, jt):
                jsl = slice(jt * 128, (jt + 1) * 128)
                c_ = jt - 4 * ib          # >=0 on diagonal chunks
                off = 128 * c_ if c_ >= 0 else 0
                osc = off if off <= 256 else 256   # keep f32r N>=256
                sc = ps.tile([128, 1024], F32, name="sc", tag="A", bufs=2)
                for h2 in range(2):
                    ho = h2 * 64
                    nc.tensor.matmul(
                        sc[:, 512 * h2 + osc:512 * h2 + 512],
                        _r(kt_s[hp][ho:ho + 64, jsl]),
                        _r(qt_s[hp][ho:ho + 64,
                                    ib * 512 + osc:ib * 512 + 512]),
                        start=True, stop=True)
                return sc, off, c_

            def warm_mm(box, n):
                # keep-warm padding: bf16 matmuls into unused psum rows
                # 96-127 of the AV accumulator; ~213ns each, they hold the
                # HAM clock gate at K=8/8 while ACT paces the softmax.
                for _ in range(n):
                    # start/stop False: ride the open AV accumulation group
                    nc.tensor.matmul(box["ot"][0][96:128, 0:512],
                                     v_s[0][:, 0:32], v_s[0][:, 0:512],
                                     start=False, stop=False,
                                     tile_position=(0, 96))

            def emit_av(hp, box, jt, p, off):
                for h2 in range(2):
                    h = 2 * hp + h2
                    nc.tensor.matmul(
                        box["ot"][h2][0:65, off:512],
                        v_s[jt][:, 65 * h:65 * h + 65],
                        p[:, 512 * h2 + off:512 * h2 + 512],
                        start=(jt == 0), stop=(jt == nj - 1))

            out = []
            for hp in range(4):
                box = {}

                def c_start(hp=hp, box=box):
                    box["ot"] = [ps.tile([128, 512], F32, name="otp",
                                         tag="B", bufs=2) for _ in range(2)]
                    box["nxt"] = emit_sc(hp, 0)
                    box["avq"] = []
                out.append(c_start)

                for jt in range(nj):
                    def c_item(hp=hp, jt=jt, box=box):
                        sc, off, c_ = box["nxt"]
                        sc3 = sc.rearrange("p (h e) -> p h e", h=2)
                        if c_ >= 0:
                            # additive causal mask (-300 below diag) pre-exp
                            nc.vector.tensor_add(
                                sc3[:, :, off:off + 128],
                                sc3[:, :, off:off + 128],
                                trif[:].rearrange("p (h e) -> p h e", h=2))
                        p = p_pool.tile([128, 1024], BF16, name="p", tag="p")
                        p3 = p.rearrange("p (h e) -> p h e", h=2)
                        nc.scalar.activation(p3[:, :, off:512],
                                             sc3[:, :, off:512], AF.Exp)
                        if jt + 1 < nj:
                            box["nxt"] = emit_sc(hp, jt + 1)
                        # AV runs one pipeline step behind its exp so the
                        # PE never waits on the ACT queue.
                        box["avq"].append((jt, p, off))
                        if len(box["avq"]) > 1:
                            emit_av(hp, box, *box["avq"].pop(0))
                            warm_mm(box, 2 if tt == NTT - 1 else 1)
                    out.append(c_item)

                def c_epi(hp=hp, box=box):
                    while box["avq"]:
                        emit_av(hp, box, *box["avq"].pop(0))
                    # epilogue: divide by the denominator row (psum row 64).
                    # partition_broadcast is the ONLY gpsimd ucode kernel in
                    # the program, so its IRAM load is paid once.
                    for h2 in range(2):
                        op = box["ot"][h2]
                        den = e_pool.tile([1, 512], F32, name="den",
                                          tag="den")
                        nc.vector.tensor_copy(den[:], op[64:65, :])
                        rden = e_pool.tile([1, 512], F32, name="rden",
                                           tag="rden")
                        nc.vector.reciprocal_approx_fast(out=rden[:],
                                                         in_=den[:])
                        rb = e_pool.tile([64, 512], F32, name="rb", tag="rb")
                        nc.gpsimd.partition_broadcast(rb[:], rden[:],
                                                      channels=64)
                        ho = h2 * 64
                        nc.vector.tensor_mul(otf[hp][ho:ho + 64, isl],
                                             op[0:64, :], rb[:])
                out.append(c_epi)
            return out

        def interleave(a, b):
            ia = ib_ = 0
            while ib_ < min(3, len(b)):     # front-load PE cover
                b[ib_]()
                ib_ += 1
            while ia < len(a) or ib_ < len(b):
                if ib_ >= len(b) or (ia < len(a)
                                     and ia * (len(b) - 3) <= (ib_ - 3)
                                     * max(1, len(a))):
                    a[ia]()
                    ia += 1
                else:
                    b[ib_]()
                    ib_ += 1

        # ---------------- schedule --------------------------------------
        xts = emit_x_dma(0)
        for f in proj_closures(0, xts, True):
            f()
        pending_py = []
        for tt in range(NTT):
            a_items = rot_closures(tt) + attn_closures(tt)
            b_items = []
            if tt + 1 < NTT:
                xts = emit_x_dma(tt + 1)
                b_items += proj_closures(tt + 1, xts, tt + 1 < NTT - 1)
            if tt == NTT - 1:
                take = [p_ for p_ in pending_py if p_[0] <= tt - 1]
            else:
                take = [p_ for p_ in pending_py if p_[0] == tt - 2]
            for p_ in take:
                pending_py.remove(p_)
                b_items.append(lambda p_=p_: py_chain(*p_))
            interleave(a_items, b_items)
            pending_py += [(tt, d) for d in range(8)]
        wps = ps.tile([128, 512], F32, name="wps", tag="B", bufs=2)
        nc.tensor.matmul(wps[96:128, 0:512], v_s[0][:, 0:32],
                         v_s[0][:, 0:512], start=True, stop=False,
                         tile_position=(0, 96))
        for pib, d in pending_py:
            py_chain(pib, d)
            for _ in range(3):
                nc.tensor.matmul(wps[96:128, 0:512], v_s[0][:, 0:32],
                                 v_s[0][:, 0:512], start=False, stop=False,
                                 tile_position=(0, 96))
        nc.tensor.matmul(wps[96:128, 0:512], v_s[0][:, 0:32],
                         v_s[0][:, 0:512], start=False, stop=True,
                         tile_position=(0, 96))
    return nc


# ---------------- host-side tables & shard prep -------------------------

def host_tables(T: int = 2048):
    n = HD // 4
    af = (1.0 / 1024) ** np.linspace(0, 1, n, dtype=np.float32)
    af = np.concatenate([af, np.zeros(n, np.float32)])  # [32]
    theta = np.outer(np.arange(T, dtype=np.float32), af)  # [T, 32]
    cosT = np.cos(theta).T.astype(np.float32)  # [32, T]
    sinT = np.sin(theta).T.astype(np.float32)
    c2 = np.tile(cosT, (4, 1))                             # [128, T]
    s2 = np.tile(np.concatenate([sinT, -sinT], 0), (2, 1))  # [128, T]
    km = np.arange(128)
    pswap = (km[:, None] == (km[None, :] ^ 32)).astype(np.float32)
    bdiag = ((km[:, None] // 64) == (km[None, :] // 64)).astype(np.float32)
    r_ = np.arange(128)[:, None]
    c_ = np.arange(128)[None, :]
    tri1 = np.where(c_ >= r_, 0.0, -300.0).astype(np.float32)
    tri = np.concatenate([tri1, tri1], axis=1)             # [128, 256]
    return {"c2": np.ascontiguousarray(c2), "s2": np.ascontiguousarray(s2),
            "pswap": pswap, "bdiag": bdiag,
            "trimask": np.ascontiguousarray(tri)}


def _bf16():
    import ml_dtypes
    return ml_dtypes.bfloat16


def core_inputs(x, wq, wk, wv, wo, core: int, T: int = 2048):
    b, g = core % 4, core // 4
    sl = slice(g * DH, (g + 1) * DH)
    m = {
        "xt": np.ascontiguousarray(np.asarray(x[b]).T.astype(np.float32)),
        "wqt": np.ascontiguousarray(np.asarray(wq)[sl, :].T.astype(np.float32)),
        "wkt": np.ascontiguousarray(np.asarray(wk)[sl, :].T.astype(np.float32)),
        "wvt": np.ascontiguousarray(np.asarray(wv)[sl, :].T.astype(np.float32)),
        "wot": np.ascontiguousarray(
            np.asarray(wo)[:, sl].T.astype(_bf16())),
    }
    m.update(host_tables(T))
    return m


_CACHE = {}


def _get_nc(T: int = 2048):
    key = ("nc", T)
    if key not in _CACHE:
        nc = bacc.Bacc("TRN2", target_bir_lowering=False, debug=False)
        build_kernel(nc, T)
        nc.compile()
        _CACHE[key] = nc
    return _CACHE[key]


def kernel(x, wq, wk, wv, wo, mask=None):
    from concourse import bass_utils
    nc = _get_nc(2048)
    in_maps = [core_inputs(x, wq, wk, wv, wo, c) for c in range(8)]
    res = bass_utils.run_bass_kernel_spmd(nc, in_maps, list(range(8)))
    outs = [np.asarray(res.results[c]["yt"]).astype(np.float32)
            for c in range(8)]
    out = np.empty((4, 2048, 1024), np.float32)
    for b in range(4):
        out[b] = (outs[b] + outs[b + 4]).T
    return out


# revision 38
# speedup vs baseline: 1.0719x; 1.0659x over previous
"""Trainium2 Bass kernel: MultiHeadAttention with QK-RMSNorm + partial rotary,
causal softmax. B=4, T=2048, D=1024, H=16, HD=64, fp32.

Sharding: 8 cores = 4 batches x 2 head-groups (8 heads each). Each core:
  - QKV projections for its batch, restricted to its 512 head-dims
  - causal attention for its 8 heads
  - partial output projection (its 512 contraction dims, all 1024 outputs)
Host sums the two head-group partials per batch (the all-reduce) and
transposes back.

v3 design: phase-interleaved, engine-specialized, single-exp scores.
  - Per 512-token block tt: QKV projections -> rotary+RMS-norm -> attention
    for i-block tt -> output projection of i-block tt. The ACT-heavy softmax
    and the DMA-heavy output store overlap the PE-heavy projections of the
    next block, keeping the PE dense so the HAM clock gate stays at K=8/8
    (fp32r at low clock is 4x slower).
  - K is normalized exactly like Q (bdiag matmul + sqrt + recip + mul), so
    exp needs no per-head scale and one [128,1024] exp op covers both heads
    of a pair (halves ACT instruction count - ACT is the attention pacer).
  - gpsimd runs ONLY partition_broadcast (softmax denominator): any second
    gpsimd ucode kernel would trigger ~6us IRAM reloads per switch.
  - ACT tables: {Square,Sqrt,Copy} + {Exp,Copy} - two loads per block max.
  - V, softmax weights, attention output and wo in bf16 (same PE rate as
    fp32r, halves SBUF/DVE cost, dodges fp32r small-N penalty on diagonal).
  - causal diagonal blocks compute/exp only the valid column strip; mask is
    an additive -300 on scores pre-exp.
"""

import numpy as np
from contextlib import ExitStack

import concourse.bass as bass
import concourse.tile as tile
import concourse.mybir as mybir
from concourse import bacc

F32 = mybir.dt.float32
BF16 = mybir.dt.bfloat16
MM_DT = mybir.dt.float32r
AF = mybir.ActivationFunctionType

D = 1024   # model dim
DH = 512   # head-group width per core (8 heads x 64)
NH = 8     # heads per core
HD = 64    # head dim
NKC = D // 128   # k-chunks over model dim
EPS = 1e-6


def _r(ap):
    return ap.bitcast(MM_DT)


def build_kernel(nc: bass.Bass, T: int = 2048):
    NTT = T // 512     # 512-wide t/i blocks
    NTS = T // 128     # 128-wide t/j chunks

    xt = nc.dram_tensor("xt", [D, T], F32, kind="ExternalInput").ap()
    wqt = nc.dram_tensor("wqt", [D, DH], F32, kind="ExternalInput").ap()
    wkt = nc.dram_tensor("wkt", [D, DH], F32, kind="ExternalInput").ap()
    wvt = nc.dram_tensor("wvt", [D, DH], F32, kind="ExternalInput").ap()
    wot = nc.dram_tensor("wot", [DH, D], BF16, kind="ExternalInput").ap()
    c2d = nc.dram_tensor("c2", [128, T], F32, kind="ExternalInput").ap()
    s2d = nc.dram_tensor("s2", [128, T], F32, kind="ExternalInput").ap()
    pswapd = nc.dram_tensor("pswap", [128, 128], F32, kind="ExternalInput").ap()
    bdiagd = nc.dram_tensor("bdiag", [128, 128], F32, kind="ExternalInput").ap()
    trid = nc.dram_tensor("trimask", [128, 256], F32, kind="ExternalInput").ap()
    yt = nc.dram_tensor("yt", [D, T], BF16, kind="ExternalOutput").ap()

    with tile.TileContext(nc) as tc, ExitStack() as ctx:
        # ---- persistent SBUF pools --------------------------------------
        qk_pool = ctx.enter_context(tc.tile_pool(name="qk", bufs=1))
        v_pool = ctx.enter_context(tc.tile_pool(name="v", bufs=1))
        ot_pool = ctx.enter_context(tc.tile_pool(name="otf", bufs=1))
        const_pool = ctx.enter_context(tc.tile_pool(name="const", bufs=1))
        w_pool = ctx.enter_context(tc.tile_pool(name="wqkv", bufs=1))
        wo_pool = ctx.enter_context(tc.tile_pool(name="wo", bufs=1))
        x_pool = ctx.enter_context(tc.tile_pool(name="xs", bufs=8))
        cs_pool = ctx.enter_context(tc.tile_pool(name="cs", bufs=1))
        t_pool = ctx.enter_context(tc.tile_pool(name="rott", bufs=2))
        p_pool = ctx.enter_context(tc.tile_pool(name="pexp", bufs=4))
        e_pool = ctx.enter_context(tc.tile_pool(name="epi", bufs=1))
        st_pool = ctx.enter_context(tc.tile_pool(name="stg3", bufs=2))
        # single PSUM pool; tags share the 8 banks across phases:
        #   A: proj accum [128,512] + scores pair [128,1024]   (2x2 banks)
        #   B: rotary pswap out + attn AV accum                (2 banks)
        #   C: rotary bdiag out + out-proj accum               (2 banks)
        ps = ctx.enter_context(tc.tile_pool(name="ps", bufs=1, space="PSUM"))

        qt_s = [qk_pool.tile([128, T], F32, name=f"qt{j}") for j in range(4)]
        kt_s = [qk_pool.tile([128, T], F32, name=f"kt{j}") for j in range(4)]
        v_s = [v_pool.tile([128, NH * 65], BF16, name=f"vt{j}")
               for j in range(NTS)]
        otf = [ot_pool.tile([128, T], BF16, name=f"otf{j}") for j in range(4)]

        # weights first: they gate the first projection matmuls.
        wq_s = [w_pool.tile([128, DH], F32, name=f"wq{k}") for k in range(NKC)]
        wk_s = [w_pool.tile([128, DH], F32, name=f"wk{k}") for k in range(NKC)]
        wv_s = [w_pool.tile([128, DH], F32, name=f"wv{k}") for k in range(NKC)]
        for k in range(NKC):
            ksl = slice(k * 128, (k + 1) * 128)
            eng = nc.sync if k % 2 == 0 else nc.scalar
            eng.dma_start(_r(wq_s[k][:]), _r(wqt[ksl, :]))
        pswap = const_pool.tile([128, 128], F32, name="pswap_s")
        bdiag = const_pool.tile([128, 128], F32, name="bdiag_s")
        trif = const_pool.tile([128, 256], F32, name="trif_s")
        nc.scalar.dma_start(_r(pswap[:]), _r(pswapd[:]))
        nc.scalar.dma_start(_r(bdiag[:]), _r(bdiagd[:]))
        for k in range(NKC):
            ksl = slice(k * 128, (k + 1) * 128)
            nc.scalar.dma_start(_r(wk_s[k][:]), _r(wkt[ksl, :]))
            nc.scalar.dma_start(_r(wv_s[k][:]), _r(wvt[ksl, :]))
        nc.scalar.dma_start(trif[:], trid[:])
        wob = [wo_pool.tile([128, D], BF16, name=f"wob{k}") for k in range(4)]
        for k4 in range(4):
            nc.scalar.dma_start(wob[k4][:], wot[k4 * 128:(k4 + 1) * 128, :])
        epsb = const_pool.tile([128, 1], F32, name="epsb")
        nc.vector.memset(epsb[:], 8.0 * EPS)
        onescb = const_pool.tile([128, NH], BF16, name="onescb")
        nc.vector.memset(onescb[:], 1.0)

        # ---------------- emission helpers ------------------------------
        def emit_x_dma(tt):
            tsl = slice(tt * 512, (tt + 1) * 512)
            xts = []
            for k in range(NKC):
                xc = x_pool.tile([128, 512], F32, name="xc", tag="xc")
                nc.gpsimd.dma_start(_r(xc[:]),
                                    _r(xt[k * 128:(k + 1) * 128, tsl]))
                xts.append(xc)
            return xts

        def py_chain(pib, dt_):
            """Out-projection of one dout chunk of i-block pib (PE filler)."""
            dsl = slice(dt_ * 128, (dt_ + 1) * 128)
            psl = slice(pib * 512, (pib + 1) * 512)
            py = ps.tile([128, 512], F32, name="py", tag="C", bufs=2)
            for k4 in range(4):
                nc.tensor.matmul(py[:], wob[k4][:, dsl], otf[k4][:, psl],
                                 start=(k4 == 0), stop=(k4 == 3))
            st = st_pool.tile([128, 512], BF16, name="st", tag="st", bufs=3)
            nc.vector.tensor_copy(st[:], py[:])
            nc.sync.dma_start(yt[dsl, psl], st[:])

        def proj_closures(tt, xts, act_copies):
            """12 PE projection k-groups for block tt (q/k/v); copies
            trail on ACT early (slack) or DVE late (exp latency)."""
            tsl = slice(tt * 512, (tt + 1) * 512)
            out = []
            for (wsrc, dsts) in ((wq_s, qt_s), (wk_s, kt_s)):
                for hp in range(4):
                    def g(wsrc=wsrc, dsts=dsts, hp=hp):
                        jsl = slice(hp * 128, (hp + 1) * 128)
                        pp = ps.tile([128, 1024], F32, name="pp", tag="A",
                                     bufs=2)
                        for k in range(NKC):
                            nc.tensor.matmul(
                                pp[:, 0:512], _r(wsrc[k][:, jsl]),
                                _r(xts[k][:]),
                                start=(k == 0), stop=(k == NKC - 1))
                        if act_copies:
                            nc.scalar.copy(_r(dsts[hp][:, tsl]), pp[:, 0:512])
                        else:
                            nc.vector.tensor_copy(_r(dsts[hp][:, tsl]),
                                                  pp[:, 0:512])
                    out.append(g)
            for ts_ in range(4):
                def g(ts_=ts_):
                    ci = tt * 4 + ts_
                    pv = ps.tile([128, 1024], F32, name="pv", tag="A", bufs=2)
                    for k in range(NKC):
                        nc.tensor.matmul(
                            pv[:, 0:512],
                            _r(xts[k][:, ts_ * 128:(ts_ + 1) * 128]),
                            _r(wv_s[k][:]),
                            start=(k == 0), stop=(k == NKC - 1))
                    v3 = v_s[ci].rearrange("p (h e) -> p h e", h=NH)
                    if act_copies:
                        nc.scalar.copy(
                            v3[:, :, 0:64],
                            pv[:, 0:512].rearrange("p (h e) -> p h e", h=NH))
                    else:
                        nc.vector.tensor_copy(
                            v3[:, :, 0:64],
                            pv[:, 0:512].rearrange("p (h e) -> p h e", h=NH))
                    nc.vector.tensor_copy(v3[:, :, 64:65],
                                          onescb[:].unsqueeze(-1))
                out.append(g)
            return out

        def rot_closures(tt):
            """Rotary + RMS-norm for q and k of block tt. Squares on DVE,
            sqrt on ACT (Square/Sqrt/Copy + Exp/Copy = 2 tables; squares
            and sqrts batch per block so table swaps stay ~2/block)."""
            tsl = slice(tt * 512, (tt + 1) * 512)
            c2t = cs_pool.tile([128, 512], F32, name="c2t", tag="c2t")
            s2t = cs_pool.tile([128, 512], F32, name="s2t", tag="s2t")
            nc.sync.dma_start(c2t[:], c2d[:, tsl])
            nc.sync.dma_start(s2t[:], s2d[:, tsl])
            out = []
            for hp in range(4):
                for nm in ("q", "k"):
                    def g(hp=hp, nm=nm):
                        dst = (qt_s if nm == "q" else kt_s)[hp]
                        sq = t_pool.tile([128, 512], F32, name="sq",
                                         tag=f"sq{nm}")
                        nc.vector.scalar_tensor_tensor(
                            _r(sq[:]), dst[:, tsl], 1.0, dst[:, tsl],
                            mybir.AluOpType.mult, mybir.AluOpType.mult)
                        xs_ = ps.tile([128, 512], F32, name="xs", tag="B",
                                      bufs=2)
                        nc.tensor.matmul(xs_[:], _r(pswap[:]),
                                         _r(dst[:, tsl]),
                                         start=True, stop=True)
                        ms = ps.tile([128, 512], F32, name="ms", tag="C",
                                     bufs=2)
                        nc.tensor.matmul(ms[:], _r(bdiag[:]), _r(sq[:]),
                                         start=True, stop=True)
                        s1 = t_pool.tile([128, 512], F32, name="s1",
                                         tag=f"s1{nm}")
                        nc.scalar.activation(s1[:], ms[:], AF.Sqrt,
                                             scale=0.125, bias=epsb[:])
                        nc.vector.reciprocal_approx_fast(out=s1[:], in_=s1[:])
                        nc.vector.tensor_mul(_r(dst[:, tsl]), dst[:, tsl],
                                             c2t[:])
                        nc.vector.tensor_mul(xs_[:], xs_[:], s2t[:])
                        nc.vector.tensor_add(_r(dst[:, tsl]), dst[:, tsl],
                                             xs_[:])
                        nc.vector.tensor_mul(_r(dst[:, tsl]), dst[:, tsl],
                                             s1[:])
                    out.append(g)
            return out

        def attn_closures(tt):
            """Software-pipelined attention for i-block tt: scores(jt+1) is
            emitted before AV(jt) so interleaved PE work covers exp."""
            ib = tt
            isl = slice(tt * 512, (tt + 1) * 512)
            nj = 4 * ib + 4
            late = tt >= 2

            def emit_sc(hp, jt):
                jsl = slice(jt * 128, (jt + 1) * 128)
                c_ = jt - 4 * ib          # >=0 on diagonal chunks
                off = 128 * c_ if c_ >= 0 else 0
                osc = off if off <= 256 else 256   # keep f32r N>=256
                sc = ps.tile([128, 1024], F32, name="sc", tag="A", bufs=2)
                for h2 in range(2):
                    ho = h2 * 64
                    nc.tensor.matmul(
                        sc[:, 512 * h2 + osc:512 * h2 + 512],
                        _r(kt_s[hp][ho:ho + 64, jsl]),
                        _r(qt_s[hp][ho:ho + 64,
                                    ib * 512 + osc:ib * 512 + 512]),
                        start=True, stop=True)
                return sc, off, c_

            def warm_mm(box, n):
                # keep-warm padding: bf16 matmuls into unused psum rows
                # 96-127 of the AV accumulator; ~213ns each, they hold the
                # HAM clock gate at K=8/8 while ACT paces the softmax.
                for _ in range(n):
                    # start/stop False: ride the open AV accumulation group
                    nc.tensor.matmul(box["ot"][0][96:128, 0:512],
                                     v_s[0][:, 0:32], v_s[0][:, 0:512],
                                     start=False, stop=False,
                                     tile_position=(0, 96))

            def emit_av(hp, box, jt, p, off):
                for h2 in range(2):
                    h = 2 * hp + h2
                    nc.tensor.matmul(
                        box["ot"][h2][0:65, off:512],
                        v_s[jt][:, 65 * h:65 * h + 65],
                        p[:, 512 * h2 + off:512 * h2 + 512],
                        start=(jt == 0), stop=(jt == nj - 1))

            out = []
            for hp in range(4):
                box = {}

                def c_start(hp=hp, box=box):
                    box["ot"] = [ps.tile([128, 512], F32, name="otp",
                                         tag="B", bufs=2) for _ in range(2)]
                    box["nxt"] = emit_sc(hp, 0)
                    box["avq"] = []
                out.append(c_start)

                for jt in range(nj):
                    def c_item(hp=hp, jt=jt, box=box):
                        sc, off, c_ = box["nxt"]
                        sc3 = sc.rearrange("p (h e) -> p h e", h=2)
                        if c_ >= 0:
                            # additive causal mask (-300 below diag) pre-exp
                            nc.vector.tensor_add(
                                sc3[:, :, off:off + 128],
                                sc3[:, :, off:off + 128],
                                trif[:].rearrange("p (h e) -> p h e", h=2))
                        p = p_pool.tile([128, 1024], BF16, name="p", tag="p")
                        p3 = p.rearrange("p (h e) -> p h e", h=2)
                        nc.scalar.activation(p3[:, :, off:512],
                                             sc3[:, :, off:512], AF.Exp)
                        if jt + 1 < nj:
                            box["nxt"] = emit_sc(hp, jt + 1)
                        # AV runs one pipeline step behind its exp so the
                        # PE never waits on the ACT queue.
                        box["avq"].append((jt, p, off))
                        if len(box["avq"]) > 1:
                            emit_av(hp, box, *box["avq"].pop(0))
                    out.append(c_item)

                def c_epi(hp=hp, box=box):
                    while box["avq"]:
                        emit_av(hp, box, *box["avq"].pop(0))
                    # epilogue: divide by the denominator row (psum row 64).
                    # partition_broadcast is the ONLY gpsimd ucode kernel in
                    # the program, so its IRAM load is paid once.
                    for h2 in range(2):
                        op = box["ot"][h2]
                        den = e_pool.tile([1, 512], F32, name="den",
                                          tag="den")
                        nc.vector.tensor_copy(den[:], op[64:65, :])
                        rden = e_pool.tile([1, 512], F32, name="rden",
                                           tag="rden")
                        nc.vector.reciprocal_approx_fast(out=rden[:],
                                                         in_=den[:])
                        rb = e_pool.tile([64, 512], F32, name="rb", tag="rb")
                        nc.gpsimd.partition_broadcast(rb[:], rden[:],
                                                      channels=64)
                        ho = h2 * 64
                        nc.vector.tensor_mul(otf[hp][ho:ho + 64, isl],
                                             op[0:64, :], rb[:])
                out.append(c_epi)
            return out

        def interleave(a, b):
            ia = ib_ = 0
            while ib_ < min(3, len(b)):     # front-load PE cover
                b[ib_]()
                ib_ += 1
            while ia < len(a) or ib_ < len(b):
                if ib_ >= len(b) or (ia < len(a)
                                     and ia * (len(b) - 3) <= (ib_ - 3)
                                     * max(1, len(a))):
                    a[ia]()
                    ia += 1
                else:
                    b[ib_]()
                    ib_ += 1

        # ---------------- schedule --------------------------------------
        xts = emit_x_dma(0)
        for f in proj_closures(0, xts, True):
            f()
        pending_py = []
        for tt in range(NTT):
            a_items = rot_closures(tt) + attn_closures(tt)
            b_items = []
            if tt + 1 < NTT:
                xts = emit_x_dma(tt + 1)
                b_items += proj_closures(tt + 1, xts, tt + 1 < NTT - 1)
            if tt == NTT - 1:
                take = [p_ for p_ in pending_py if p_[0] <= tt - 1]
            else:
                take = [p_ for p_ in pending_py if p_[0] == tt - 2]
            for p_ in take:
                pending_py.remove(p_)
                b_items.append(lambda p_=p_: py_chain(*p_))
            interleave(a_items, b_items)
            pending_py += [(tt, d) for d in range(8)]
        wps = ps.tile([128, 512], F32, name="wps", tag="B", bufs=2)
        nc.tensor.matmul(wps[96:128, 0:512], v_s[0][:, 0:32],
                         v_s[0][:, 0:512], start=True, stop=False,
                         tile_position=(0, 96))
        for pib, d in pending_py:
            py_chain(pib, d)
            for _ in range(3):
                nc.tensor.matmul(wps[96:128, 0:512], v_s[0][:, 0:32],
                                 v_s[0][:, 0:512], start=False, stop=False,
                                 tile_position=(0, 96))
        nc.tensor.matmul(wps[96:128, 0:512], v_s[0][:, 0:32],
                         v_s[0][:, 0:512], start=False, stop=True,
                         tile_position=(0, 96))
    return nc


# ---------------- host-side tables & shard prep -------------------------

def host_tables(T: int = 2048):
    n = HD // 4
    af = (1.0 / 1024) ** np.linspace(0, 1, n, dtype=np.float32)
    af = np.concatenate([af, np.zeros(n, np.float32)])  # [32]
    theta = np.outer(np.arange(T, dtype=np.float32), af)  # [T, 32]
    cosT = np.cos(theta).T.astype(np.float32)  # [32, T]
    sinT = np.sin(theta).T.astype(np.float32)
    c2 = np.tile(cosT, (4, 1))                             # [128, T]
    s2 = np.tile(np.concatenate([sinT, -sinT], 0), (2, 1))  # [128, T]
    km = np.arange(128)
    pswap = (km[:, None] == (km[None, :] ^ 32)).astype(np.float32)
    bdiag = ((km[:, None] // 64) == (km[None, :] // 64)).astype(np.float32)
    r_ = np.arange(128)[:, None]
    c_ = np.arange(128)[None, :]
    tri1 = np.where(c_ >= r_, 0.0, -300.0).astype(np.float32)
    tri = np.concatenate([tri1, tri1], axis=1)             # [128, 256]
    return {"c2": np.ascontiguousarray(c2), "s2": np.ascontiguousarray(s2),
            "pswap": pswap, "bdiag": bdiag,
            "trimask": np.ascontiguousarray(tri)}


def _bf16():
    import ml_dtypes
    return ml_dtypes.bfloat16


def core_inputs(x, wq, wk, wv, wo, core: int, T: int = 2048):
    b, g = core % 4, core // 4
    sl = slice(g * DH, (g + 1) * DH)
    m = {
        "xt": np.ascontiguousarray(np.asarray(x[b]).T.astype(np.float32)),
        "wqt": np.ascontiguousarray(np.asarray(wq)[sl, :].T.astype(np.float32)),
        "wkt": np.ascontiguousarray(np.asarray(wk)[sl, :].T.astype(np.float32)),
        "wvt": np.ascontiguousarray(np.asarray(wv)[sl, :].T.astype(np.float32)),
        "wot": np.ascontiguousarray(
            np.asarray(wo)[:, sl].T.astype(_bf16())),
    }
    m.update(host_tables(T))
    return m


_CACHE = {}


def _get_nc(T: int = 2048):
    key = ("nc", T)
    if key not in _CACHE:
        nc = bacc.Bacc("TRN2", target_bir_lowering=False, debug=False)
        build_kernel(nc, T)
        nc.compile()
        _CACHE[key] = nc
    return _CACHE[key]


def kernel(x, wq, wk, wv, wo, mask=None):
    from concourse import bass_utils
    nc = _get_nc(2048)
    in_maps = [core_inputs(x, wq, wk, wv, wo, c) for c in range(8)]
    res = bass_utils.run_bass_kernel_spmd(nc, in_maps, list(range(8)))
    outs = [np.asarray(res.results[c]["yt"]).astype(np.float32)
            for c in range(8)]
    out = np.empty((4, 2048, 1024), np.float32)
    for b in range(4):
        out[b] = (outs[b] + outs[b + 4]).T
    return out


# revision 39
# speedup vs baseline: 1.0784x; 1.0061x over previous
"""Trainium2 Bass kernel: MultiHeadAttention with QK-RMSNorm + partial rotary,
causal softmax. B=4, T=2048, D=1024, H=16, HD=64, fp32.

Sharding: 8 cores = 4 batches x 2 head-groups (8 heads each). Each core:
  - QKV projections for its batch, restricted to its 512 head-dims
  - causal attention for its 8 heads
  - partial output projection (its 512 contraction dims, all 1024 outputs)
Host sums the two head-group partials per batch (the all-reduce) and
transposes back.

v3 design: phase-interleaved, engine-specialized, single-exp scores.
  - Per 512-token block tt: QKV projections -> rotary+RMS-norm -> attention
    for i-block tt -> output projection of i-block tt. The ACT-heavy softmax
    and the DMA-heavy output store overlap the PE-heavy projections of the
    next block, keeping the PE dense so the HAM clock gate stays at K=8/8
    (fp32r at low clock is 4x slower).
  - K is normalized exactly like Q (bdiag matmul + sqrt + recip + mul), so
    exp needs no per-head scale and one [128,1024] exp op covers both heads
    of a pair (halves ACT instruction count - ACT is the attention pacer).
  - gpsimd runs ONLY partition_broadcast (softmax denominator): any second
    gpsimd ucode kernel would trigger ~6us IRAM reloads per switch.
  - ACT tables: {Square,Sqrt,Copy} + {Exp,Copy} - two loads per block max.
  - V, softmax weights, attention output and wo in bf16 (same PE rate as
    fp32r, halves SBUF/DVE cost, dodges fp32r small-N penalty on diagonal).
  - causal diagonal blocks compute/exp only the valid column strip; mask is
    an additive -300 on scores pre-exp.
"""

import numpy as np
from contextlib import ExitStack

import concourse.bass as bass
import concourse.tile as tile
import concourse.mybir as mybir
from concourse import bacc

F32 = mybir.dt.float32
BF16 = mybir.dt.bfloat16
MM_DT = mybir.dt.float32r
AF = mybir.ActivationFunctionType

D = 1024   # model dim
DH = 512   # head-group width per core (8 heads x 64)
NH = 8     # heads per core
HD = 64    # head dim
NKC = D // 128   # k-chunks over model dim
EPS = 1e-6


def _r(ap):
    return ap.bitcast(MM_DT)


def build_kernel(nc: bass.Bass, T: int = 2048):
    NTT = T // 512     # 512-wide t/i blocks
    NTS = T // 128     # 128-wide t/j chunks

    xt = nc.dram_tensor("xt", [D, T], F32, kind="ExternalInput").ap()
    wqt = nc.dram_tensor("wqt", [D, DH], F32, kind="ExternalInput").ap()
    wkt = nc.dram_tensor("wkt", [D, DH], F32, kind="ExternalInput").ap()
    wvt = nc.dram_tensor("wvt", [D, DH], F32, kind="ExternalInput").ap()
    wot = nc.dram_tensor("wot", [DH, D], BF16, kind="ExternalInput").ap()
    c2d = nc.dram_tensor("c2", [128, T], F32, kind="ExternalInput").ap()
    s2d = nc.dram_tensor("s2", [128, T], F32, kind="ExternalInput").ap()
    pswapd = nc.dram_tensor("pswap", [128, 128], F32, kind="ExternalInput").ap()
    bdiagd = nc.dram_tensor("bdiag", [128, 128], F32, kind="ExternalInput").ap()
    trid = nc.dram_tensor("trimask", [128, 256], F32, kind="ExternalInput").ap()
    yt = nc.dram_tensor("yt", [D, T], BF16, kind="ExternalOutput").ap()

    with tile.TileContext(nc) as tc, ExitStack() as ctx:
        # ---- persistent SBUF pools --------------------------------------
        qk_pool = ctx.enter_context(tc.tile_pool(name="qk", bufs=1))
        v_pool = ctx.enter_context(tc.tile_pool(name="v", bufs=1))
        ot_pool = ctx.enter_context(tc.tile_pool(name="otf", bufs=1))
        const_pool = ctx.enter_context(tc.tile_pool(name="const", bufs=1))
        w_pool = ctx.enter_context(tc.tile_pool(name="wqkv", bufs=1))
        wo_pool = ctx.enter_context(tc.tile_pool(name="wo", bufs=1))
        x_pool = ctx.enter_context(tc.tile_pool(name="xs", bufs=8))
        cs_pool = ctx.enter_context(tc.tile_pool(name="cs", bufs=1))
        t_pool = ctx.enter_context(tc.tile_pool(name="rott", bufs=2))
        p_pool = ctx.enter_context(tc.tile_pool(name="pexp", bufs=4))
        e_pool = ctx.enter_context(tc.tile_pool(name="epi", bufs=1))
        st_pool = ctx.enter_context(tc.tile_pool(name="stg3", bufs=2))
        # single PSUM pool; tags share the 8 banks across phases:
        #   A: proj accum [128,512] + scores pair [128,1024]   (2x2 banks)
        #   B: rotary pswap out + attn AV accum                (2 banks)
        #   C: rotary bdiag out + out-proj accum               (2 banks)
        ps = ctx.enter_context(tc.tile_pool(name="ps", bufs=1, space="PSUM"))

        qt_s = [qk_pool.tile([128, T], F32, name=f"qt{j}") for j in range(4)]
        kt_s = [qk_pool.tile([128, T], F32, name=f"kt{j}") for j in range(4)]
        v_s = [v_pool.tile([128, NH * 65], BF16, name=f"vt{j}")
               for j in range(NTS)]
        otf = [ot_pool.tile([128, T], BF16, name=f"otf{j}") for j in range(4)]

        # weights first: they gate the first projection matmuls.
        wq_s = [w_pool.tile([128, DH], F32, name=f"wq{k}") for k in range(NKC)]
        wk_s = [w_pool.tile([128, DH], F32, name=f"wk{k}") for k in range(NKC)]
        wv_s = [w_pool.tile([128, DH], F32, name=f"wv{k}") for k in range(NKC)]
        for k in range(NKC):
            ksl = slice(k * 128, (k + 1) * 128)
            eng = nc.sync if k % 2 == 0 else nc.scalar
            eng.dma_start(_r(wq_s[k][:]), _r(wqt[ksl, :]))
        pswap = const_pool.tile([128, 128], F32, name="pswap_s")
        bdiag = const_pool.tile([128, 128], F32, name="bdiag_s")
        trif = const_pool.tile([128, 256], F32, name="trif_s")
        nc.scalar.dma_start(_r(pswap[:]), _r(pswapd[:]))
        nc.scalar.dma_start(_r(bdiag[:]), _r(bdiagd[:]))
        for k in range(NKC):
            ksl = slice(k * 128, (k + 1) * 128)
            nc.scalar.dma_start(_r(wk_s[k][:]), _r(wkt[ksl, :]))
            nc.scalar.dma_start(_r(wv_s[k][:]), _r(wvt[ksl, :]))
        nc.scalar.dma_start(trif[:], trid[:])
        wob = [wo_pool.tile([128, D], BF16, name=f"wob{k}") for k in range(4)]
        for k4 in range(4):
            nc.scalar.dma_start(wob[k4][:], wot[k4 * 128:(k4 + 1) * 128, :])
        epsb = const_pool.tile([128, 1], F32, name="epsb")
        nc.vector.memset(epsb[:], 8.0 * EPS)
        onesf = const_pool.tile([128, 1], F32, name="onesf")
        nc.vector.memset(onesf[:], 1.0)
        # ACT-ordering tokens: sqrt(window tt) waits last exp(tt-1) and
        # exp(tt) waits last sqrt(tt), so the ACT queue never ping-pongs
        # between the Sqrt and Exp tables (1.28us ACT_TABLE_LOAD each).
        tstate = {}
        onescb = const_pool.tile([128, NH], BF16, name="onescb")
        nc.vector.memset(onescb[:], 1.0)

        # ---------------- emission helpers ------------------------------
        def emit_x_dma(tt):
            tsl = slice(tt * 512, (tt + 1) * 512)
            xts = []
            for k in range(NKC):
                xc = x_pool.tile([128, 512], F32, name="xc", tag="xc")
                nc.gpsimd.dma_start(_r(xc[:]),
                                    _r(xt[k * 128:(k + 1) * 128, tsl]))
                xts.append(xc)
            return xts

        def py_chain(pib, dt_):
            """Out-projection of one dout chunk of i-block pib (PE filler)."""
            dsl = slice(dt_ * 128, (dt_ + 1) * 128)
            psl = slice(pib * 512, (pib + 1) * 512)
            py = ps.tile([128, 512], F32, name="py", tag="C", bufs=2)
            for k4 in range(4):
                nc.tensor.matmul(py[:], wob[k4][:, dsl], otf[k4][:, psl],
                                 start=(k4 == 0), stop=(k4 == 3))
            st = st_pool.tile([128, 512], BF16, name="st", tag="st", bufs=3)
            nc.vector.tensor_copy(st[:], py[:])
            nc.sync.dma_start(yt[dsl, psl], st[:])

        def proj_closures(tt, xts, act_copies):
            """12 PE projection k-groups for block tt (q/k/v); copies
            trail on ACT early (slack) or DVE late (exp latency)."""
            tsl = slice(tt * 512, (tt + 1) * 512)
            out = []
            for (wsrc, dsts) in ((wq_s, qt_s), (wk_s, kt_s)):
                for hp in range(4):
                    def g(wsrc=wsrc, dsts=dsts, hp=hp):
                        jsl = slice(hp * 128, (hp + 1) * 128)
                        pp = ps.tile([128, 1024], F32, name="pp", tag="A",
                                     bufs=2)
                        for k in range(NKC):
                            nc.tensor.matmul(
                                pp[:, 0:512], _r(wsrc[k][:, jsl]),
                                _r(xts[k][:]),
                                start=(k == 0), stop=(k == NKC - 1))
                        if act_copies:
                            nc.scalar.copy(_r(dsts[hp][:, tsl]), pp[:, 0:512])
                        else:
                            nc.vector.tensor_copy(_r(dsts[hp][:, tsl]),
                                                  pp[:, 0:512])
                    out.append(g)
            for ts_ in range(4):
                def g(ts_=ts_):
                    ci = tt * 4 + ts_
                    pv = ps.tile([128, 1024], F32, name="pv", tag="A", bufs=2)
                    for k in range(NKC):
                        nc.tensor.matmul(
                            pv[:, 0:512],
                            _r(xts[k][:, ts_ * 128:(ts_ + 1) * 128]),
                            _r(wv_s[k][:]),
                            start=(k == 0), stop=(k == NKC - 1))
                    v3 = v_s[ci].rearrange("p (h e) -> p h e", h=NH)
                    if act_copies:
                        nc.scalar.copy(
                            v3[:, :, 0:64],
                            pv[:, 0:512].rearrange("p (h e) -> p h e", h=NH))
                    else:
                        nc.vector.tensor_copy(
                            v3[:, :, 0:64],
                            pv[:, 0:512].rearrange("p (h e) -> p h e", h=NH))
                    nc.vector.tensor_copy(v3[:, :, 64:65],
                                          onescb[:].unsqueeze(-1))
                out.append(g)
            return out

        def rot_closures(tt):
            """Rotary + RMS-norm for q and k of block tt. Squares on DVE,
            sqrt on ACT (Square/Sqrt/Copy + Exp/Copy = 2 tables; squares
            and sqrts batch per block so table swaps stay ~2/block)."""
            tsl = slice(tt * 512, (tt + 1) * 512)
            c2t = cs_pool.tile([128, 512], F32, name="c2t", tag="c2t")
            s2t = cs_pool.tile([128, 512], F32, name="s2t", tag="s2t")
            nc.sync.dma_start(c2t[:], c2d[:, tsl])
            nc.sync.dma_start(s2t[:], s2d[:, tsl])
            out = []

            def mk_epsw():
                if tstate.get("tok1") is not None:
                    epsw = e_pool.tile([128, 1], F32, name="epsw", tag="epsw",
                                       bufs=2)
                    nc.vector.scalar_tensor_tensor(
                        epsw[:], tstate["tok1"][:], 0.0, epsb[:],
                        mybir.AluOpType.mult, mybir.AluOpType.add)
                    tstate["epsw"] = epsw
                else:
                    tstate["epsw"] = epsb
            out.append(mk_epsw)
            for hp in range(4):
                for nm in ("q", "k"):
                    def g(hp=hp, nm=nm):
                        dst = (qt_s if nm == "q" else kt_s)[hp]
                        sq = t_pool.tile([128, 512], F32, name="sq",
                                         tag=f"sq{nm}")
                        nc.vector.scalar_tensor_tensor(
                            _r(sq[:]), dst[:, tsl], 1.0, dst[:, tsl],
                            mybir.AluOpType.mult, mybir.AluOpType.mult)
                        xs_ = ps.tile([128, 512], F32, name="xs", tag="B",
                                      bufs=2)
                        nc.tensor.matmul(xs_[:], _r(pswap[:]),
                                         _r(dst[:, tsl]),
                                         start=True, stop=True)
                        ms = ps.tile([128, 512], F32, name="ms", tag="C",
                                     bufs=2)
                        nc.tensor.matmul(ms[:], _r(bdiag[:]), _r(sq[:]),
                                         start=True, stop=True)
                        s1 = t_pool.tile([128, 512], F32, name="s1",
                                         tag=f"s1{nm}")
                        acc = None
                        if hp == 3 and nm == "k":
                            acc = e_pool.tile([128, 1], F32, name="tok2",
                                              tag="tok2", bufs=2)
                            tstate["tok2"] = acc
                        nc.scalar.activation(s1[:], ms[:], AF.Sqrt,
                                             scale=0.125,
                                             bias=tstate["epsw"][:],
                                             accum_out=(acc[:] if acc is not
                                                        None else None))
                        nc.vector.reciprocal_approx_fast(out=s1[:], in_=s1[:])
                        nc.vector.tensor_mul(_r(dst[:, tsl]), dst[:, tsl],
                                             c2t[:])
                        nc.vector.tensor_mul(xs_[:], xs_[:], s2t[:])
                        nc.vector.tensor_add(_r(dst[:, tsl]), dst[:, tsl],
                                             xs_[:])
                        nc.vector.tensor_mul(_r(dst[:, tsl]), dst[:, tsl],
                                             s1[:])
                    out.append(g)
            return out

        def attn_closures(tt):
            """Software-pipelined attention for i-block tt: scores(jt+1) is
            emitted before AV(jt) so interleaved PE work covers exp."""
            ib = tt
            isl = slice(tt * 512, (tt + 1) * 512)
            nj = 4 * ib + 4
            late = tt >= 2

            def emit_sc(hp, jt):
                jsl = slice(jt * 128, (jt + 1) * 128)
                c_ = jt - 4 * ib          # >=0 on diagonal chunks
                off = 128 * c_ if c_ >= 0 else 0
                osc = off if off <= 256 else 256   # keep f32r N>=256
                sc = ps.tile([128, 1024], F32, name="sc", tag="A", bufs=2)
                for h2 in range(2):
                    ho = h2 * 64
                    nc.tensor.matmul(
                        sc[:, 512 * h2 + osc:512 * h2 + 512],
                        _r(kt_s[hp][ho:ho + 64, jsl]),
                        _r(qt_s[hp][ho:ho + 64,
                                    ib * 512 + osc:ib * 512 + 512]),
                        start=True, stop=True)
                return sc, off, c_

            def warm_mm(box, n):
                # keep-warm padding: bf16 matmuls into unused psum rows
                # 96-127 of the AV accumulator; ~213ns each, they hold the
                # HAM clock gate at K=8/8 while ACT paces the softmax.
                for _ in range(n):
                    # start/stop False: ride the open AV accumulation group
                    nc.tensor.matmul(box["ot"][0][96:128, 0:512],
                                     v_s[0][:, 0:32], v_s[0][:, 0:512],
                                     start=False, stop=False,
                                     tile_position=(0, 96))

            def emit_av(hp, box, jt, p, off):
                for h2 in range(2):
                    h = 2 * hp + h2
                    nc.tensor.matmul(
                        box["ot"][h2][0:65, off:512],
                        v_s[jt][:, 65 * h:65 * h + 65],
                        p[:, 512 * h2 + off:512 * h2 + 512],
                        start=(jt == 0), stop=(jt == nj - 1))

            out = []
            for hp in range(4):
                box = {}

                def c_start(hp=hp, box=box):
                    if hp == 0:
                        if tstate.get("tok2") is not None:
                            sone = e_pool.tile([128, 1], F32, name="sone",
                                               tag="sone", bufs=2)
                            nc.vector.scalar_tensor_tensor(
                                sone[:], tstate["tok2"][:], 0.0, onesf[:],
                                mybir.AluOpType.mult, mybir.AluOpType.add)
                            tstate["sone"] = sone
                        else:
                            tstate["sone"] = onesf
                    box["ot"] = [ps.tile([128, 512], F32, name="otp",
                                         tag="B", bufs=2) for _ in range(2)]
                    box["nxt"] = emit_sc(hp, 0)
                    box["avq"] = []
                out.append(c_start)

                for jt in range(nj):
                    def c_item(hp=hp, jt=jt, box=box):
                        sc, off, c_ = box["nxt"]
                        sc3 = sc.rearrange("p (h e) -> p h e", h=2)
                        if c_ >= 0:
                            # additive causal mask (-300 below diag) pre-exp
                            nc.vector.tensor_add(
                                sc3[:, :, off:off + 128],
                                sc3[:, :, off:off + 128],
                                trif[:].rearrange("p (h e) -> p h e", h=2))
                        p = p_pool.tile([128, 1024], BF16, name="p", tag="p")
                        p3 = p.rearrange("p (h e) -> p h e", h=2)
                        acc = None
                        if hp == 3 and jt == nj - 1:
                            acc = e_pool.tile([128, 1], F32, name="tok1",
                                              tag="tok1", bufs=2)
                            tstate["tok1"] = acc
                        nc.scalar.activation(p3[:, :, off:512],
                                             sc3[:, :, off:512], AF.Exp,
                                             scale=tstate["sone"][:],
                                             accum_out=(acc[:] if acc is not
                                                        None else None))
                        if jt + 1 < nj:
                            box["nxt"] = emit_sc(hp, jt + 1)
                        # AV runs one pipeline step behind its exp so the
                        # PE never waits on the ACT queue.
                        box["avq"].append((jt, p, off))
                        if len(box["avq"]) > 1:
                            emit_av(hp, box, *box["avq"].pop(0))
                    out.append(c_item)

                def c_epi(hp=hp, box=box):
                    while box["avq"]:
                        emit_av(hp, box, *box["avq"].pop(0))
                    # epilogue: divide by the denominator row (psum row 64).
                    # partition_broadcast is the ONLY gpsimd ucode kernel in
                    # the program, so its IRAM load is paid once.
                    for h2 in range(2):
                        op = box["ot"][h2]
                        den = e_pool.tile([1, 512], F32, name="den",
                                          tag="den")
                        nc.vector.tensor_copy(den[:], op[64:65, :])
                        rden = e_pool.tile([1, 512], F32, name="rden",
                                           tag="rden")
                        nc.vector.reciprocal_approx_fast(out=rden[:],
                                                         in_=den[:])
                        rb = e_pool.tile([64, 512], F32, name="rb", tag="rb")
                        nc.gpsimd.partition_broadcast(rb[:], rden[:],
                                                      channels=64)
                        ho = h2 * 64
                        nc.vector.tensor_mul(otf[hp][ho:ho + 64, isl],
                                             op[0:64, :], rb[:])
                out.append(c_epi)
            return out

        def interleave(a, b):
            ia = ib_ = 0
            while ib_ < min(3, len(b)):     # front-load PE cover
                b[ib_]()
                ib_ += 1
            while ia < len(a) or ib_ < len(b):
                if ib_ >= len(b) or (ia < len(a)
                                     and ia * (len(b) - 3) <= (ib_ - 3)
                                     * max(1, len(a))):
                    a[ia]()
                    ia += 1
                else:
                    b[ib_]()
                    ib_ += 1

        # ---------------- schedule --------------------------------------
        xts = emit_x_dma(0)
        for f in proj_closures(0, xts, True):
            f()
        pending_py = []
        for tt in range(NTT):
            a_items = rot_closures(tt) + attn_closures(tt)
            b_items = []
            if tt + 1 < NTT:
                xts = emit_x_dma(tt + 1)
                b_items += proj_closures(tt + 1, xts, tt + 1 < NTT - 1)
            if tt == NTT - 1:
                take = [p_ for p_ in pending_py if p_[0] <= tt - 1]
            else:
                take = [p_ for p_ in pending_py if p_[0] == tt - 2]
            for p_ in take:
                pending_py.remove(p_)
                b_items.append(lambda p_=p_: py_chain(*p_))
            interleave(a_items, b_items)
            pending_py += [(tt, d) for d in range(8)]
        wps = ps.tile([128, 512], F32, name="wps", tag="B", bufs=2)
        nc.tensor.matmul(wps[96:128, 0:512], v_s[0][:, 0:32],
                         v_s[0][:, 0:512], start=True, stop=False,
                         tile_position=(0, 96))
        for pib, d in pending_py:
            py_chain(pib, d)
            for _ in range(3):
                nc.tensor.matmul(wps[96:128, 0:512], v_s[0][:, 0:32],
                                 v_s[0][:, 0:512], start=False, stop=False,
                                 tile_position=(0, 96))
        nc.tensor.matmul(wps[96:128, 0:512], v_s[0][:, 0:32],
                         v_s[0][:, 0:512], start=False, stop=True,
                         tile_position=(0, 96))
    return nc


# ---------------- host-side tables & shard prep -------------------------

def host_tables(T: int = 2048):
    n = HD // 4
    af = (1.0 / 1024) ** np.linspace(0, 1, n, dtype=np.float32)
    af = np.concatenate([af, np.zeros(n, np.float32)])  # [32]
    theta = np.outer(np.arange(T, dtype=np.float32), af)  # [T, 32]
    cosT = np.cos(theta).T.astype(np.float32)  # [32, T]
    sinT = np.sin(theta).T.astype(np.float32)
    c2 = np.tile(cosT, (4, 1))                             # [128, T]
    s2 = np.tile(np.concatenate([sinT, -sinT], 0), (2, 1))  # [128, T]
    km = np.arange(128)
    pswap = (km[:, None] == (km[None, :] ^ 32)).astype(np.float32)
    bdiag = ((km[:, None] // 64) == (km[None, :] // 64)).astype(np.float32)
    r_ = np.arange(128)[:, None]
    c_ = np.arange(128)[None, :]
    tri1 = np.where(c_ >= r_, 0.0, -300.0).astype(np.float32)
    tri = np.concatenate([tri1, tri1], axis=1)             # [128, 256]
    return {"c2": np.ascontiguousarray(c2), "s2": np.ascontiguousarray(s2),
            "pswap": pswap, "bdiag": bdiag,
            "trimask": np.ascontiguousarray(tri)}


def _bf16():
    import ml_dtypes
    return ml_dtypes.bfloat16


def core_inputs(x, wq, wk, wv, wo, core: int, T: int = 2048):
    b, g = core % 4, core // 4
    sl = slice(g * DH, (g + 1) * DH)
    m = {
        "xt": np.ascontiguousarray(np.asarray(x[b]).T.astype(np.float32)),
        "wqt": np.ascontiguousarray(np.asarray(wq)[sl, :].T.astype(np.float32)),
        "wkt": np.ascontiguousarray(np.asarray(wk)[sl, :].T.astype(np.float32)),
        "wvt": np.ascontiguousarray(np.asarray(wv)[sl, :].T.astype(np.float32)),
        "wot": np.ascontiguousarray(
            np.asarray(wo)[:, sl].T.astype(_bf16())),
    }
    m.update(host_tables(T))
    return m


_CACHE = {}


def _get_nc(T: int = 2048):
    key = ("nc", T)
    if key not in _CACHE:
        nc = bacc.Bacc("TRN2", target_bir_lowering=False, debug=False)
        build_kernel(nc, T)
        nc.compile()
        _CACHE[key] = nc
    return _CACHE[key]


def kernel(x, wq, wk, wv, wo, mask=None):
    from concourse import bass_utils
    nc = _get_nc(2048)
    in_maps = [core_inputs(x, wq, wk, wv, wo, c) for c in range(8)]
    res = bass_utils.run_bass_kernel_spmd(nc, in_maps, list(range(8)))
    outs = [np.asarray(res.results[c]["yt"]).astype(np.float32)
            for c in range(8)]
    out = np.empty((4, 2048, 1024), np.float32)
    for b in range(4):
        out[b] = (outs[b] + outs[b + 4]).T
    return out
